# revision 38
# baseline (speedup 1.0000x reference)
"""CPI-MPNN (molecule MPNN + protein CNN + FC head) Trainium2 kernel.

Self-contained: hardcodes all shapes. Shards the batch (128) across 8
NeuronCores (16 samples each), replicates the small weights.

Strategy (from 228us baseline to ~158us):
  - Conv tower in fp8 (final rel err ~5e-3 vs 2e-2 gate): conv0 as
    regular fp8 matmuls with host-stacked im2col (taps {0,1} packed into
    K=100), conv1/conv2 as fp8 DoubleRow matmuls (2 taps per pass; the
    pair stride must be 16-aligned, so activations are stored twice:
    [C, 2, SEGA] where slot1 is the same data shifted one column, slot0
    written by the ACT engine, slot1 duplicated by a DVE byte copy).
  - Quantization scales are powers of two folded into weights/biases;
    the conv2 output is unscaled once on the pooled [feat, M] tile.
  - Software-pipelined sample slots: slot k emits conv0(k+1), conv1(k),
    conv2(k-1) as one contiguous fp8 PE run (one DR mode switch per
    slot), so each stage's ACT/DVE activation writes land a full slot
    before their consumer.
  - MPNN in bf16: neighbor sums computed directly transposed (lhsT role
    swap, no PE transposes), W_i@fbonds re-streamed into the iteration
    PSUM (no binput copy or DVE add), atom stage feature-major with the
    molecule mean as a DVE reduce, W_o contraction batched 4 molecules
    per weight load. Microstages are spread across the sample slots.
  - PE-clock warmup matmuls during the initial DMA wait (the tensor
    engine p-state needs ~3us of busy time to reach 2.4 GHz), split DMA
    queues (SP/ACT HWDGE + gpsimd SWDGE for late weights), and the
    emb half of fc0 pre-accumulated before the conv tail.
"""

import os

import numpy as np
from contextlib import ExitStack

import concourse.bass as bass
import concourse.tile as tile
from concourse import bacc, mybir
from concourse.bass_utils import run_bass_kernel_spmd

F32 = mybir.dt.float32
BF16 = mybir.dt.bfloat16
F8 = mybir.dt.float8e4
AF = mybir.ActivationFunctionType
ALU = mybir.AluOpType
DR = mybir.MatmulPerfMode.DoubleRow
AX = mybir.AxisListType

# model dims
H = 200
B, NA, NB = 128, 48, 96
L = 1000

NCORES = 8
M = B // NCORES          # samples per core (16)
SEGA = 1008              # 3 pad + 1000 + 5 tail zeros (16-aligned)
PAD = 3
NCH = 500                # conv free-dim chunk (2 per sample)
GM = 4                   # molecules per MPNN group

# quantization scales (powers of two; ranges measured on the reference
# data distribution with >=4x overflow headroom in fp8e4m3)
A0 = 2.0 ** 9            # x0 stored scale
G0 = 2.0 ** 8            # conv0 weight scale
A1 = 2.0 ** 8            # x1 stored scale
G1 = 2.0 ** 8            # conv1 weight scale
A2 = 2.0 ** 8            # x2 stored scale
G2 = 2.0 ** 8            # conv2 weight scale
S1 = A1 / (A0 * G0)      # conv0 ACT scale
S2 = A2 / (A1 * G1)      # conv1 ACT scale
S3 = 1.0 / (A2 * G2)     # conv2 output unscale

_CACHE = {}


def _build_nc():
    nc = bacc.Bacc("TRN2", target_bir_lowering=False, debug=False)

    # ---- DRAM inputs (per core) ----
    d_pvt = [nc.dram_tensor(f"pvt{s}", [100, 2, SEGA], F8,
                            kind="ExternalInput")
             for s in range(M)]
    d_fbt = nc.dram_tensor("fbt", [50, M, 96], BF16, kind="ExternalInput")
    d_cat1 = nc.dram_tensor("cat1", [40, M, 48], BF16, kind="ExternalInput")
    d_abt = nc.dram_tensor("abt", [96, M, 96], BF16, kind="ExternalInput")
    d_aat = nc.dram_tensor("aat", [96, M, 48], BF16, kind="ExternalInput")

    d_wi = nc.dram_tensor("wi", [50, 200], BF16, kind="ExternalInput")
    d_wh1 = nc.dram_tensor("wh1", [128, 200], BF16, kind="ExternalInput")
    d_wh2 = nc.dram_tensor("wh2", [72, 200], BF16, kind="ExternalInput")
    d_wo1 = nc.dram_tensor("wo1", [40, 200], BF16, kind="ExternalInput")
    d_wo2 = nc.dram_tensor("wo2", [128, 200], BF16, kind="ExternalInput")
    d_wo3 = nc.dram_tensor("wo3", [72, 200], BF16, kind="ExternalInput")

    d_w0dr = nc.dram_tensor("w0dr", [100, 2, 96], F8, kind="ExternalInput")
    d_b0 = nc.dram_tensor("b0", [96, 1], F32, kind="ExternalInput")
    d_w1dr = nc.dram_tensor("w1dr", [96, 3, 2, 128], F8, kind="ExternalInput")
    d_b1 = nc.dram_tensor("b1", [128, 1], F32, kind="ExternalInput")
    d_w2a = nc.dram_tensor("w2a", [128, 4, 2, 128], F8, kind="ExternalInput")
    d_w2b = nc.dram_tensor("w2b", [128, 4, 2, 80], F8, kind="ExternalInput")
    d_b2a = nc.dram_tensor("b2a", [128, 1], F32, kind="ExternalInput")
    d_b2b = nc.dram_tensor("b2b", [72, 1], F32, kind="ExternalInput")

    d_fc0 = [nc.dram_tensor(f"fc0{k}", [dim, 200], BF16, kind="ExternalInput")
             for k, dim in (("a", 128), ("b", 72), ("c", 128), ("d", 72))]
    d_fc0ba = nc.dram_tensor("fc0ba", [128, 1], F32, kind="ExternalInput")
    d_fc0bb = nc.dram_tensor("fc0bb", [72, 1], F32, kind="ExternalInput")
    d_fc1a = nc.dram_tensor("fc1a", [128, 100], BF16, kind="ExternalInput")
    d_fc1b = nc.dram_tensor("fc1b", [72, 100], BF16, kind="ExternalInput")
    d_fc1bias = nc.dram_tensor("fc1bias", [100, 1], F32, kind="ExternalInput")
    d_fc2w = nc.dram_tensor("fc2w", [100, 1], BF16, kind="ExternalInput")
    d_fc2b = nc.dram_tensor("fc2b", [1, 1], F32, kind="ExternalInput")

    d_out = nc.dram_tensor("out", [1, M], F32, kind="ExternalOutput")

    with tile.TileContext(nc) as tc, ExitStack() as ctx:
        cst = ctx.enter_context(tc.tile_pool(name="cst", bufs=1))
        sbs = ctx.enter_context(tc.tile_pool(name="sbs", bufs=1))
        tmp = ctx.enter_context(tc.tile_pool(name="tmp", bufs=1))
        xp = ctx.enter_context(tc.tile_pool(name="xp", bufs=1))
        pp = ctx.enter_context(tc.tile_pool(name="pp", bufs=1, space="PSUM"))

        def const_tile(dram, shape, dtype=BF16, eng=None):
            t = cst.tile(shape, dtype, tag=dram.name)
            (eng or nc.sync).dma_start(t[:], dram.ap())
            return t

        wu = tmp.tile([128, 512], BF16, tag="wu")
        nc.gpsimd.memset(wu[:].bitcast(F32), 0.0)

        fbt_g, abt_g, aat_g, cat1_g = {}, {}, {}, {}

        def mpnn_dma_fb(g, eng):
            t = cst.tile([50, GM * 96], BF16, tag=f"fbt{g}")
            eng.dma_start(t[:].rearrange("p (m i) -> p m i", i=96),
                          d_fbt.ap()[:, GM * g:GM * (g + 1), :])
            fbt_g[g] = t

        def mpnn_dma(g, eng):
            mpnn_dma_fb(g, eng)
            t = cst.tile([96, GM * 96], BF16, tag=f"abt{g}")
            eng.dma_start(t[:].rearrange("p (m i) -> p m i", i=96),
                          d_abt.ap()[:, GM * g:GM * (g + 1), :])
            abt_g[g] = t
            t = cst.tile([96, GM * 48], BF16, tag=f"aat{g}")
            eng.dma_start(t[:].rearrange("p (m i) -> p m i", i=48),
                          d_aat.ap()[:, GM * g:GM * (g + 1), :])
            aat_g[g] = t
            t = cst.tile([40, GM * 48], BF16, tag=f"cat1{g}")
            eng.dma_start(t[:].rearrange("p (m i) -> p m i", i=48),
                          d_cat1.ap()[:, GM * g:GM * (g + 1), :])
            cat1_g[g] = t

        # ACT queue: conv0/conv1 weights + the first protein samples so
        # the conv pipeline can start immediately.
        w0dr_t = cst.tile([100, 2 * 96], F8, tag="w0dr")
        nc.scalar.dma_start(
            w0dr_t[:].rearrange("p (b o) -> p b o", b=2), d_w0dr.ap())
        b0_t = const_tile(d_b0, [96, 1], F32, eng=nc.scalar)
        x0_bufs = {}

        def pvt_dma(s):
            t = xp.tile([100, 2 * SEGA], F8, tag="x0", bufs=5)
            eng = nc.scalar if s % 2 == 0 else nc.sync
            eng.dma_start(t[:].rearrange("p (a s) -> p a s", a=2),
                          d_pvt[s].ap())
            x0_bufs[s] = t

        wi_t = const_tile(d_wi, [50, 200])
        mpnn_dma_fb(0, nc.sync)
        t0 = xp.tile([100, 2 * SEGA], F8, tag="x0", bufs=5)
        nc.scalar.dma_start(
            t0[0:50, :].rearrange("p (a s) -> p a s", a=2),
            d_pvt[0].ap()[0:50, :, :])
        nc.sync.dma_start(
            t0[50:100, :].rearrange("p (a s) -> p a s", a=2),
            d_pvt[0].ap()[50:100, :, :])
        x0_bufs[0] = t0
        pvt_dma(1)
        pvt_dma(3)
        abt0 = cst.tile([96, GM * 96], BF16, tag="abt0")
        nc.sync.dma_start(abt0[:].rearrange("p (m i) -> p m i", i=96),
                          d_abt.ap()[:, 0:GM, :])
        abt_g[0] = abt0
        aat0 = cst.tile([96, GM * 48], BF16, tag="aat0")
        nc.sync.dma_start(aat0[:].rearrange("p (m i) -> p m i", i=48),
                          d_aat.ap()[:, 0:GM, :])
        aat_g[0] = aat0
        cat10 = cst.tile([40, GM * 48], BF16, tag="cat10")
        nc.sync.dma_start(cat10[:].rearrange("p (m i) -> p m i", i=48),
                          d_cat1.ap()[:, 0:GM, :])
        cat1_g[0] = cat10
        w1dr_t = cst.tile([96, 3 * 2 * 128], F8, tag="w1dr")
        nc.scalar.dma_start(
            w1dr_t[:].rearrange("p (a b o) -> p a b o", a=3, b=2), d_w1dr.ap())
        b1_t = const_tile(d_b1, [128, 1], F32, eng=nc.scalar)
        pvt_dma(2)
        w2a_t = cst.tile([128, 4 * 2 * 128], F8, tag="w2a")
        nc.scalar.dma_start(
            w2a_t[:].rearrange("p (a b o) -> p a b o", a=4, b=2), d_w2a.ap())
        w2b_t = cst.tile([128, 4 * 2 * 80], F8, tag="w2b")
        nc.scalar.dma_start(
            w2b_t[:].rearrange("p (a b o) -> p a b o", a=4, b=2), d_w2b.ap())
        b2a_t = const_tile(d_b2a, [128, 1], F32, eng=nc.scalar)
        b2b_t = const_tile(d_b2b, [72, 1], F32, eng=nc.scalar)

        # Late-needed weights (W_o, FC, groups 2-3) ride the slower
        # gpsimd SWDGE so the two HWDGE queues stay clear for pvt.
        wh1_t = const_tile(d_wh1, [128, 200])
        wh2_t = const_tile(d_wh2, [72, 200])
        mpnn_dma(1, nc.sync)
        wo1_t = const_tile(d_wo1, [40, 200], eng=nc.gpsimd)
        wo2_t = const_tile(d_wo2, [128, 200], eng=nc.gpsimd)
        wo3_t = const_tile(d_wo3, [72, 200], eng=nc.gpsimd)
        mpnn_dma(2, nc.gpsimd)
        mpnn_dma(3, nc.gpsimd)
        fc0_t = [const_tile(d, [dim, 200], eng=nc.gpsimd) for d, dim in
                 zip(d_fc0, (128, 72, 128, 72))]
        fc0ba_t = const_tile(d_fc0ba, [128, 1], F32, eng=nc.gpsimd)
        fc0bb_t = const_tile(d_fc0bb, [72, 1], F32, eng=nc.gpsimd)
        fc1a_t = const_tile(d_fc1a, [128, 100], eng=nc.gpsimd)
        fc1b_t = const_tile(d_fc1b, [72, 100], eng=nc.gpsimd)
        fc1bias_t = const_tile(d_fc1bias, [100, 1], F32, eng=nc.gpsimd)
        fc2w_t = const_tile(d_fc2w, [100, 1], eng=nc.gpsimd)
        fc2b_t = const_tile(d_fc2b, [1, 1], F32, eng=nc.gpsimd)

        # static outputs of the two towers, feature-major [feat, M]
        embT1 = sbs.tile([128, M], BF16, tag="embT1")
        embT2 = sbs.tile([72, M], BF16, tag="embT2")
        prT1p = sbs.tile([128, M], F32, tag="prT1p")
        prT2p = sbs.tile([72, M], F32, tag="prT2p")
        prT1 = sbs.tile([128, M], BF16, tag="prT1")
        prT2 = sbs.tile([72, M], BF16, tag="prT2")

        # ================= per-molecule MPNN (bf16) =================
        msg_t, nt_t = {}, {}
        natg_t = {}

        binp_t = {}

        def emit_binput(m):
            g, r = m // GM, m % GM
            fb_m = fbt_g[g][:, r * 96:(r + 1) * 96]
            ps = pp.tile([96, 200], F32, tag="mp", bufs=3)
            nc.tensor.matmul(ps[:], fb_m, wi_t[:], start=True, stop=True)
            binp = sbs.tile([96, 200], BF16, tag=f"binp{m}")
            nc.scalar.copy(binp[:], ps[:])
            msg = sbs.tile([96, 200], BF16, tag=f"msg{m}")
            nc.vector.tensor_scalar(msg[:], ps[:], 0.0, None, op0=ALU.max)
            msg_t[m] = msg
            binp_t[m] = binp

        def emit_iter_nei(m):
            # nT[h, i] = sum_j msg[j, h] * ab[j, i], feature-major halves
            # packed into one PSUM bank, one fused copy out.
            g, r = m // GM, m % GM
            ab_m = abt_g[g][:, r * 96:(r + 1) * 96]
            msg = msg_t[m]
            psn = pp.tile([128, 192], F32, tag="tp", bufs=2)
            nc.tensor.matmul(psn[0:128, 0:96], msg[:, 0:128], ab_m,
                             start=True, stop=True)
            nc.tensor.matmul(psn[0:72, 96:192], msg[:, 128:200], ab_m,
                             start=True, stop=True, skip_group_check=True)
            nt = tmp.tile([128, 192], BF16, tag="nt", bufs=4)
            nc.vector.tensor_copy(nt[0:128, 0:96], psn[0:128, 0:96])
            nc.scalar.copy(nt[0:72, 96:192], psn[0:72, 96:192])
            nt_t[m] = nt

        def emit_iter_upd(m):
            # msg = relu(nT.T @ W_h + binput); the binput add runs on
            # the DVE (PE is the bottleneck, DVE/ACT have slack)
            nt = nt_t.pop(m)
            msg = msg_t[m]
            psH = pp.tile([96, 200], F32, tag="mp", bufs=3)
            nc.tensor.matmul(psH[:], nt[0:128, 0:96], wh1_t[:],
                             start=True, stop=False)
            nc.tensor.matmul(psH[:], nt[0:72, 96:192], wh2_t[:],
                             start=False, stop=True)
            tm = tmp.tile([96, 200], F32, tag="mtmp", bufs=3)
            nc.vector.tensor_add(tm[:], psH[:], binp_t[m][:])
            nc.scalar.activation(msg[:], tm[:], AF.Relu)

        def emit_atom_pre(m):
            g, r = m // GM, m % GM
            aa_m = aat_g[g][:, r * 48:(r + 1) * 48]
            msg = msg_t.pop(m)
            binp_t.pop(m, None)
            if r == 0:
                natg_new = tmp.tile([128, GM * 96], BF16, tag="natg", bufs=2)
                natg_t[g] = natg_new
            natg = natg_t[g]
            pst = pp.tile([128, 96], F32, tag="tp", bufs=2)
            nc.tensor.matmul(pst[0:128, 0:48], msg[:, 0:128], aa_m,
                             start=True, stop=True)
            nc.tensor.matmul(pst[0:72, 48:96], msg[:, 128:200], aa_m,
                             start=True, stop=True, skip_group_check=True)
            nc.scalar.copy(natg[:, r * 96:(r + 1) * 96], pst[:])

        def emit_atom_group(g):
            natg = natg_t.pop(g)
            natv = natg[:].rearrange("p (m i) -> p m i", i=96)
            c1_g = cat1_g[g]
            for lo, hi, wsl, emb, scr in (
                    (0, 128, slice(0, 128), embT1, "ah1"),
                    (128, 200, slice(128, 200), embT2, "ah2")):
                n = hi - lo
                psU = pp.tile([n, GM * 48], F32, tag="mp", bufs=3)
                nc.tensor.matmul(psU[:], wo1_t[:, wsl], c1_g[:],
                                 start=True, stop=False)
                nc.tensor.matmul(
                    psU[:].rearrange("p (m i) -> p m i", i=48),
                    wo2_t[:, wsl], natv[0:128, :, 0:48],
                    start=False, stop=False)
                nc.tensor.matmul(
                    psU[:].rearrange("p (m i) -> p m i", i=48),
                    wo3_t[:, wsl], natv[0:72, :, 48:96],
                    start=False, stop=True)
                ah = tmp.tile([n, GM * 48], BF16, tag=scr, bufs=2)
                nc.scalar.activation(ah[:], psU[:], AF.Relu)
                red = tmp.tile([n, GM], F32, tag=scr + "r", bufs=2)
                nc.vector.reduce_sum(
                    red[:], ah[:].rearrange("p (m a) -> p m a", a=48), axis=AX.X)
                nc.scalar.mul(emb[:, GM * g:GM * (g + 1)], red[:], 1.0 / 48)

        # ================= per-sample protein conv tower =================
        x1_t, x2_t = {}, {}

        def dup_ap(t, nparts, off):
            # [nparts, 2, NCH] DR pair AP over a duplicated [nparts,
            # 2*SEGA] tile (slot1 = same data shifted left one column).
            a = t[:, off:off + NCH]
            return bass.AP(a.tensor, a.offset,
                           [[2 * SEGA, nparts], [SEGA, 2], [1, NCH]])

        def dup_write(t, nparts, off, ps, bias, scale):
            # ACT writes slot0 = relu(scale*psum + bias) in fp8; DVE
            # copies the shifted byte range into slot1.
            nc.scalar.activation(t[:, off:off + NCH], ps[:], AF.Relu,
                                 bias=bias, scale=scale)
            nc.vector.tensor_copy(t[:, SEGA + off - 1:SEGA + off - 1 + NCH],
                                  t[:, off - 1:off - 1 + NCH])

        def emit_conv0(s):
            if s + 4 < M:
                pvt_dma(s + 4)
            x0 = x0_bufs[s]
            x1 = xp.tile([96, 2 * SEGA], F8, tag="x1", bufs=3)
            x1w = x1[:].bitcast(F32)
            nc.gpsimd.memset(x1w[:, 0:1], 0.0)
            nc.gpsimd.memset(x1w[:, 250:253], 0.0)
            nc.gpsimd.memset(x1w[:, 502:504], 0.0)
            w0v = w0dr_t[:].rearrange("p (b o) -> p b o", b=2)
            for c in range(2):
                off = PAD + c * NCH
                a = x0[:, 2 + c * NCH:2 + c * NCH + NCH]
                rhs = bass.AP(a.tensor, a.offset,
                              [[2 * SEGA, 100], [SEGA, 2], [1, NCH]])
                ps = pp.tile([96, NCH], F32, tag="cv", bufs=3)
                nc.tensor.matmul(ps[:], w0v, rhs, start=True, stop=True,
                                 perf_mode=DR)
                dup_write(x1, 96, off, ps, b0_t[:], S1)
            x1_t[s] = x1

        def emit_conv1(s):
            x1 = x1_t.pop(s)
            x2 = xp.tile([128, 2 * SEGA], F8, tag="x2", bufs=3)
            x2w = x2[:].bitcast(F32)
            nc.gpsimd.memset(x2w[:, 0:1], 0.0)
            nc.gpsimd.memset(x2w[:, 250:253], 0.0)
            nc.gpsimd.memset(x2w[:, 502:504], 0.0)
            w1v = w1dr_t[:].rearrange("p (a b o) -> p a b o", a=3, b=2)
            for c in range(2):
                off = PAD + c * NCH
                ps = pp.tile([128, NCH], F32, tag="cv", bufs=3)
                for b in range(3):
                    nc.tensor.matmul(ps[:], w1v[:, b, :, :],
                                     dup_ap(x1, 96, off + 2 * b - 2),
                                     start=(b == 0), stop=(b == 2),
                                     perf_mode=DR)
                dup_write(x2, 128, off, ps, b1_t[:], S2)
            x2_t[s] = x2

        def emit_conv2(s):
            x2 = x2_t.pop(s)
            mxA = tmp.tile([128, 2], F32, tag="mxA", bufs=3)
            mxB = tmp.tile([72, 2], F32, tag="mxB", bufs=3)
            w2av = w2a_t[:].rearrange("p (a b o) -> p a b o", a=4, b=2)
            w2bv = w2b_t[:].rearrange("p (a b o) -> p a b o", a=4, b=2)
            for c in range(2):
                off = PAD + c * NCH
                psA = pp.tile([128, NCH], F32, tag="cv", bufs=3)
                for b in range(4):
                    nc.tensor.matmul(psA[:], w2av[:, b, :, :],
                                     dup_ap(x2, 128, off + 2 * b - 3),
                                     start=(b == 0), stop=(b == 3),
                                     perf_mode=DR)
                nc.vector.reduce_max(mxA[:, c:c + 1], psA[:], axis=AX.X)
                psB = pp.tile([80, NCH], F32, tag="cv", bufs=3)
                for b in range(4):
                    nc.tensor.matmul(psB[:], w2bv[:, b, :, :],
                                     dup_ap(x2, 128, off + 2 * b - 3),
                                     start=(b == 0), stop=(b == 3),
                                     perf_mode=DR)
                nc.vector.reduce_max(mxB[:, c:c + 1], psB[0:72, :], axis=AX.X)
            nc.vector.reduce_max(prT1p[:, s:s + 1], mxA[:], axis=AX.X)
            nc.vector.reduce_max(prT2p[:, s:s + 1], mxB[:], axis=AX.X)
            nc.scalar.activation(prT1[:, s:s + 1], prT1p[:, s:s + 1],
                                 AF.Relu, bias=b2a_t[:], scale=S3)
            nc.scalar.activation(prT2[:, s:s + 1], prT2p[:, s:s + 1],
                                 AF.Relu, bias=b2b_t[:], scale=S3)

        # ============== software-pipelined emission schedule ==============
        # slot k: conv0(k+1) | conv1(k) | conv2(k-1) contiguous, with two
        # MPNN microstages of group k//4 around the conv run.
        def mp_stage(g, idx):
            mols = [GM * g + r for r in range(GM)]
            if idx == 0:
                for m in mols:
                    emit_binput(m)
            elif idx in (1, 3):
                for m in mols:
                    emit_iter_nei(m)
            elif idx in (2, 4):
                for m in mols:
                    emit_iter_upd(m)
            elif idx == 5:
                for m in mols:
                    emit_atom_pre(m)
            elif idx == 6:
                emit_atom_group(g)

        # PE clock warmup: the tensor engine p-state ramps with busy
        # time, and the first ~12us are DMA-bound with an idle PE. Run
        # throwaway matmuls (one PSUM allocation, pure in-engine WAW) so
        # the clock is ramped when real work arrives.
        # warm with the same fp8 DoubleRow profile as the real conv work
        # so the clock governor's conservative period burns during the
        # DMA wait instead of on the first samples.
        wu8 = wu[:].bitcast(F8)
        wuw = bass.AP(wu8.tensor, wu8.offset,
                      [[1024, 128], [128, 2], [1, 128]])
        wur = bass.AP(wu8.tensor, wu8.offset,
                      [[1024, 128], [512, 2], [1, 500]])
        pswu = pp.tile([128, 500], F32, tag="cv", bufs=3)
        for _ in range(13):
            nc.tensor.matmul(pswu[:], wuw, wur, start=True, stop=True,
                             perf_mode=DR)

        def keep_warm(n):
            # standalone weight loads: PE-busy filler (keeps the clock
            # p-state up) with no PSUM or cross-engine dependencies.
            for _ in range(n):
                nc.tensor.ldweights(wu[:, 0:128])

        mp_stage(0, 0)
        emit_conv0(0)
        keep_warm(10)
        fc_state = {}

        def emit_fc_early():
            # emb-dependent half of fc0 right after the last atom group;
            # the prT half joins at the very end.
            ps0a = pp.tile([128, M], F32, tag="mp", bufs=3)
            nc.tensor.matmul(ps0a[:], fc0_t[0][:, 0:128], embT1[:],
                             start=True, stop=False)
            nc.tensor.matmul(ps0a[:], fc0_t[1][:, 0:128], embT2[:],
                             start=False, stop=False)
            ps0b = pp.tile([72, M], F32, tag="mp", bufs=3)
            nc.tensor.matmul(ps0b[:], fc0_t[0][:, 128:200], embT1[:],
                             start=True, stop=False)
            nc.tensor.matmul(ps0b[:], fc0_t[1][:, 128:200], embT2[:],
                             start=False, stop=False)
            fc_state["a"] = ps0a
            fc_state["b"] = ps0b

        for k in range(M):
            g, phase = k // GM, k % GM
            if k + 1 < M:
                emit_conv0(k + 1)
            mp_stage(g, 2 * phase + 1)
            if k < 2:
                keep_warm(8)
            emit_conv1(k)
            if k - 1 >= 0:
                emit_conv2(k - 1)
            if 2 * phase + 2 <= 6:
                mp_stage(g, 2 * phase + 2)
            if phase == GM - 1 and g + 1 < 4:
                mp_stage(g + 1, 0)
            if k == M - 1:
                emit_fc_early()
        emit_conv2(M - 1)

        # ================= FC head (prT half + tail) =================
        ps0a, ps0b = fc_state.pop("a"), fc_state.pop("b")
        nc.tensor.matmul(ps0a[:], fc0_t[2][:, 0:128], prT1[:],
                         start=False, stop=False)
        nc.tensor.matmul(ps0a[:], fc0_t[3][:, 0:128], prT2[:],
                         start=False, stop=True)
        h0a = tmp.tile([128, M], BF16, tag="h0a")
        nc.scalar.activation(h0a[:], ps0a[:], AF.Relu, bias=fc0ba_t[:])
        nc.tensor.matmul(ps0b[:], fc0_t[2][:, 128:200], prT1[:],
                         start=False, stop=False)
        nc.tensor.matmul(ps0b[:], fc0_t[3][:, 128:200], prT2[:],
                         start=False, stop=True)
        h0b = tmp.tile([72, M], BF16, tag="h0b")
        nc.scalar.activation(h0b[:], ps0b[:], AF.Relu, bias=fc0bb_t[:])

        ps1 = pp.tile([100, M], F32, tag="tp", bufs=2)
        nc.tensor.matmul(ps1[:], fc1a_t[:], h0a[:], start=True, stop=False)
        nc.tensor.matmul(ps1[:], fc1b_t[:], h0b[:], start=False, stop=True)
        h1 = tmp.tile([100, M], BF16, tag="h1")
        nc.scalar.activation(h1[:], ps1[:], AF.Relu, bias=fc1bias_t[:])

        ps2 = pp.tile([1, M], F32, tag="tp", bufs=2)
        nc.tensor.matmul(ps2[:], fc2w_t[:], h1[:], start=True, stop=True)
        outsb = tmp.tile([1, M], F32, tag="outsb")
        nc.scalar.add(outsb[:], ps2[:], fc2b_t[:, 0:1])
        nc.sync.dma_start(d_out.ap(), outsb[:])

    nc.compile()
    return nc


def _prep(inputs):
    """Host preprocessing: returns the 8 per-core in_maps."""
    import ml_dtypes
    f32 = np.float32
    bf16 = ml_dtypes.bfloat16
    f8 = ml_dtypes.float8_e4m3fn

    fatoms = np.asarray(inputs["fatoms"], f32)
    fbonds = np.asarray(inputs["fbonds"], f32)
    agraph = np.asarray(inputs["agraph"])
    bgraph = np.asarray(inputs["bgraph"])
    pseq = np.asarray(inputs["protein_seq"])
    W_i = np.asarray(inputs["W_i"], f32)
    W_h = np.asarray(inputs["W_h"], f32)
    W_o_w = np.asarray(inputs["W_o_w"], f32)
    W_o_b = np.asarray(inputs["W_o_b"], f32)
    embp = np.asarray(inputs["embed_protein"], f32)

    # protein embeddings, channel-major, pre-scaled for fp8
    pvT = np.ascontiguousarray((embp * A0)[pseq].transpose(0, 2, 1))  # (B,50,L)

    # adjacency one-hots (counts; contraction-dim-major for lhsT/rhs use)
    ar = np.arange(B)[:, None, None]
    cntB = np.zeros((B, NB, NB), f32)
    np.add.at(cntB, (ar, np.arange(NB)[None, :, None], bgraph), 1.0)
    abt = np.ascontiguousarray(cntB.transpose(0, 2, 1))        # (B, j, i)
    cntA = np.zeros((B, NA, NB), f32)
    np.add.at(cntA, (ar, np.arange(NA)[None, :, None], agraph), 1.0)
    aat = np.ascontiguousarray(cntA.transpose(0, 2, 1))        # (B, j, a)

    fbT = fbonds.transpose(0, 2, 1)                            # (B, 50, 96)
    faT = fatoms.transpose(0, 2, 1)                            # (B, 39, 48)
    cat1 = np.concatenate([faT, np.ones((B, 1, NA), f32)], axis=1)  # (B,40,48)

    conv_w = [np.asarray(inputs[f"conv{i}_w"], f32) for i in range(3)]
    conv_b = [np.asarray(inputs[f"conv{i}_b"], f32) for i in range(3)]

    # conv0 as one DoubleRow block: pair rows 0-49 = (tap0 -> tap1),
    # rows 50-99 = (tap2 -> zero)
    w0 = conv_w[0] * G0                                        # (96, 50, 3)
    w0dr = np.zeros((100, 2, 96), f32)
    w0dr[0:50, 0, :] = w0[:, :, 0].T
    w0dr[50:100, 0, :] = w0[:, :, 2].T
    w0dr[0:50, 1, :] = w0[:, :, 1].T

    def dr_pack(w, nblk, O):
        # w: (Ow, C, K) -> (C, nblk, 2, O) blocks of tap pairs; O may be
        # zero-padded past Ow so the DR pair stride stays 16-aligned
        Ow, C, K = w.shape
        out = np.zeros((C, nblk, 2, O), f32)
        for b in range(nblk):
            for i in range(2):
                t = 2 * b + i
                if t < K:
                    out[:, b, i, 0:Ow] = w[:, :, t].T
        return out

    w1dr = dr_pack(conv_w[1] * G1, 3, 128)                     # (96,3,2,128)
    w2 = conv_w[2] * G2                                        # (200,128,7)
    w2adr = dr_pack(w2[0:128], 4, 128)                         # (128,4,2,128)
    w2bdr = dr_pack(w2[128:200], 4, 80)        # zero-padded 72->80 for DR

    fcw = [np.asarray(inputs[f"fc{i}_w"], f32) for i in range(3)]
    fcb = [np.asarray(inputs[f"fc{i}_b"], f32) for i in range(3)]

    wo1 = np.zeros((40, 200), f32)
    wo1[:39] = W_o_w[0:39]
    wo1[39] = W_o_b

    shared = {
        "wi": W_i.astype(bf16),
        "wh1": W_h[0:128].astype(bf16), "wh2": W_h[128:200].astype(bf16),
        "wo1": wo1.astype(bf16),
        "wo2": W_o_w[39:167].astype(bf16), "wo3": W_o_w[167:239].astype(bf16),
        "w0dr": w0dr.astype(f8),
        "b0": (conv_b[0] * A1).reshape(96, 1).astype(f32),
        "w1dr": w1dr.astype(f8),
        "b1": (conv_b[1] * A2).reshape(128, 1).astype(f32),
        "w2a": w2adr.astype(f8), "w2b": w2bdr.astype(f8),
        "b2a": conv_b[2][0:128].reshape(128, 1).astype(f32),
        "b2b": conv_b[2][128:200].reshape(72, 1).astype(f32),
        "fc0a": fcw[0][0:128].astype(bf16),
        "fc0b": fcw[0][128:200].astype(bf16),
        "fc0c": fcw[0][200:328].astype(bf16),
        "fc0d": fcw[0][328:400].astype(bf16),
        "fc0ba": fcb[0][0:128].reshape(128, 1).astype(f32),
        "fc0bb": fcb[0][128:200].reshape(72, 1).astype(f32),
        "fc1a": fcw[1][0:128].astype(bf16),
        "fc1b": fcw[1][128:200].astype(bf16),
        "fc1bias": fcb[1].reshape(100, 1).astype(f32),
        "fc2w": fcw[2].astype(bf16),
        "fc2b": fcb[2].reshape(1, 1).astype(f32),
    }
    shared = {k: np.ascontiguousarray(v) for k, v in shared.items()}

    # protein activations for the DR conv0: slot0 rows 0-49 = x0 with
    # x[l] at col 3+l (tap0 reads col 2+n -> x[n-1]); rows 50-99 at col
    # 1+l (-> x[n+1], tap2). slot1 = slot0 shifted left one column.
    pvt_pad = np.zeros((B, 100, 2, SEGA), f8)
    pv8 = pvT.astype(f8)
    pvt_pad[:, 0:50, 0, 3:3 + L] = pv8
    pvt_pad[:, 50:100, 0, 1:1 + L] = pv8
    pvt_pad[:, 0:50, 1, 2:2 + L] = pv8
    pvt_pad[:, 50:100, 1, 0:0 + L] = pv8

    in_maps = []
    for c in range(NCORES):
        lo = c * M
        im = dict(shared)
        for s in range(M):
            im[f"pvt{s}"] = np.ascontiguousarray(pvt_pad[lo + s])
        im["fbt"] = np.ascontiguousarray(
            fbT[lo:lo + M].transpose(1, 0, 2)).astype(bf16)
        im["cat1"] = np.ascontiguousarray(
            cat1[lo:lo + M].transpose(1, 0, 2)).astype(bf16)
        im["abt"] = np.ascontiguousarray(
            abt[lo:lo + M].transpose(1, 0, 2)).astype(bf16)
        im["aat"] = np.ascontiguousarray(
            aat[lo:lo + M].transpose(1, 0, 2)).astype(bf16)
        in_maps.append(im)
    return in_maps


def get_nc():
    if "nc" not in _CACHE:
        _CACHE["nc"] = _build_nc()
    return _CACHE["nc"]


def kernel(**inputs) -> np.ndarray:
    nc = get_nc()
    in_maps = _prep(inputs)
    res = run_bass_kernel_spmd(nc, in_maps, core_ids=list(range(NCORES)))
    outs = [res.results[c]["out"].reshape(M, 1) for c in range(NCORES)]
    return np.concatenate(outs, axis=0).astype(np.float32)


# revision 39
# speedup vs baseline: 1.0261x; 1.0261x over previous
"""CPI-MPNN (molecule MPNN + protein CNN + FC head) Trainium2 kernel.

Self-contained: hardcodes all shapes. Shards the batch (128) across 8
NeuronCores (16 samples each), replicates the small weights.

Strategy (from 228us baseline to ~158us):
  - Conv tower in fp8 (final rel err ~5e-3 vs 2e-2 gate): conv0 as
    regular fp8 matmuls with host-stacked im2col (taps {0,1} packed into
    K=100), conv1/conv2 as fp8 DoubleRow matmuls (2 taps per pass; the
    pair stride must be 16-aligned, so activations are stored twice:
    [C, 2, SEGA] where slot1 is the same data shifted one column, slot0
    written by the ACT engine, slot1 duplicated by a DVE byte copy).
  - Quantization scales are powers of two folded into weights/biases;
    the conv2 output is unscaled once on the pooled [feat, M] tile.
  - Software-pipelined sample slots: slot k emits conv0(k+1), conv1(k),
    conv2(k-1) as one contiguous fp8 PE run (one DR mode switch per
    slot), so each stage's ACT/DVE activation writes land a full slot
    before their consumer.
  - MPNN in bf16: neighbor sums computed directly transposed (lhsT role
    swap, no PE transposes), W_i@fbonds re-streamed into the iteration
    PSUM (no binput copy or DVE add), atom stage feature-major with the
    molecule mean as a DVE reduce, W_o contraction batched 4 molecules
    per weight load. Microstages are spread across the sample slots.
  - PE-clock warmup matmuls during the initial DMA wait (the tensor
    engine p-state needs ~3us of busy time to reach 2.4 GHz), split DMA
    queues (SP/ACT HWDGE + gpsimd SWDGE for late weights), and the
    emb half of fc0 pre-accumulated before the conv tail.
"""

import os

import numpy as np
from contextlib import ExitStack

import concourse.bass as bass
import concourse.tile as tile
from concourse import bacc, mybir
from concourse.bass_utils import run_bass_kernel_spmd

F32 = mybir.dt.float32
BF16 = mybir.dt.bfloat16
F8 = mybir.dt.float8e4
AF = mybir.ActivationFunctionType
ALU = mybir.AluOpType
DR = mybir.MatmulPerfMode.DoubleRow
AX = mybir.AxisListType

# model dims
H = 200
B, NA, NB = 128, 48, 96
L = 1000

NCORES = 8
M = B // NCORES          # samples per core (16)
SEGA = 1008              # 3 pad + 1000 + 5 tail zeros (16-aligned)
PAD = 3
NCH = 500                # conv free-dim chunk (2 per sample)
GM = 4                   # molecules per MPNN group

# quantization scales (powers of two; ranges measured on the reference
# data distribution with >=4x overflow headroom in fp8e4m3)
A0 = 2.0 ** 9            # x0 stored scale
G0 = 2.0 ** 8            # conv0 weight scale
A1 = 2.0 ** 8            # x1 stored scale
G1 = 2.0 ** 8            # conv1 weight scale
A2 = 2.0 ** 8            # x2 stored scale
G2 = 2.0 ** 8            # conv2 weight scale
S1 = A1 / (A0 * G0)      # conv0 ACT scale
S2 = A2 / (A1 * G1)      # conv1 ACT scale
S3 = 1.0 / (A2 * G2)     # conv2 output unscale

_CACHE = {}


def _build_nc():
    nc = bacc.Bacc("TRN2", target_bir_lowering=False, debug=False)

    # ---- DRAM inputs (per core) ----
    d_pvt = [nc.dram_tensor(f"pvt{s}", [100, 2, SEGA], F8,
                            kind="ExternalInput")
             for s in range(M)]
    d_fbt = nc.dram_tensor("fbt", [50, M, 96], BF16, kind="ExternalInput")
    d_cat1 = nc.dram_tensor("cat1", [40, M, 48], BF16, kind="ExternalInput")
    d_abt = nc.dram_tensor("abt", [96, M, 96], BF16, kind="ExternalInput")
    d_aat = nc.dram_tensor("aat", [96, M, 48], BF16, kind="ExternalInput")

    d_wi = nc.dram_tensor("wi", [50, 200], BF16, kind="ExternalInput")
    d_wh1 = nc.dram_tensor("wh1", [128, 200], BF16, kind="ExternalInput")
    d_wh2 = nc.dram_tensor("wh2", [72, 200], BF16, kind="ExternalInput")
    d_wo1 = nc.dram_tensor("wo1", [40, 200], BF16, kind="ExternalInput")
    d_wo2 = nc.dram_tensor("wo2", [128, 200], BF16, kind="ExternalInput")
    d_wo3 = nc.dram_tensor("wo3", [72, 200], BF16, kind="ExternalInput")

    d_w0dr = nc.dram_tensor("w0dr", [100, 2, 96], F8, kind="ExternalInput")
    d_b0 = nc.dram_tensor("b0", [96, 1], F32, kind="ExternalInput")
    d_w1dr = nc.dram_tensor("w1dr", [96, 3, 2, 128], F8, kind="ExternalInput")
    d_b1 = nc.dram_tensor("b1", [128, 1], F32, kind="ExternalInput")
    d_w2a = nc.dram_tensor("w2a", [128, 4, 2, 128], F8, kind="ExternalInput")
    d_w2b = nc.dram_tensor("w2b", [128, 4, 2, 80], F8, kind="ExternalInput")
    d_b2a = nc.dram_tensor("b2a", [128, 1], F32, kind="ExternalInput")
    d_b2b = nc.dram_tensor("b2b", [72, 1], F32, kind="ExternalInput")

    d_fc0 = [nc.dram_tensor(f"fc0{k}", [dim, 200], BF16, kind="ExternalInput")
             for k, dim in (("a", 128), ("b", 72), ("c", 128), ("d", 72))]
    d_fc0ba = nc.dram_tensor("fc0ba", [128, 1], F32, kind="ExternalInput")
    d_fc0bb = nc.dram_tensor("fc0bb", [72, 1], F32, kind="ExternalInput")
    d_fc1a = nc.dram_tensor("fc1a", [128, 100], BF16, kind="ExternalInput")
    d_fc1b = nc.dram_tensor("fc1b", [72, 100], BF16, kind="ExternalInput")
    d_fc1bias = nc.dram_tensor("fc1bias", [100, 1], F32, kind="ExternalInput")
    d_fc2w = nc.dram_tensor("fc2w", [100, 1], BF16, kind="ExternalInput")
    d_fc2b = nc.dram_tensor("fc2b", [1, 1], F32, kind="ExternalInput")

    d_out = nc.dram_tensor("out", [1, M], F32, kind="ExternalOutput")

    with tile.TileContext(nc) as tc, ExitStack() as ctx:
        cst = ctx.enter_context(tc.tile_pool(name="cst", bufs=1))
        sbs = ctx.enter_context(tc.tile_pool(name="sbs", bufs=1))
        tmp = ctx.enter_context(tc.tile_pool(name="tmp", bufs=1))
        xp = ctx.enter_context(tc.tile_pool(name="xp", bufs=1))
        pp = ctx.enter_context(tc.tile_pool(name="pp", bufs=1, space="PSUM"))

        def const_tile(dram, shape, dtype=BF16, eng=None):
            t = cst.tile(shape, dtype, tag=dram.name)
            (eng or nc.sync).dma_start(t[:], dram.ap())
            return t

        wu = tmp.tile([128, 512], BF16, tag="wu")
        nc.gpsimd.memset(wu[:].bitcast(F32), 0.0)

        fbt_g, abt_g, aat_g, cat1_g = {}, {}, {}, {}

        def mpnn_dma_fb(g, eng):
            t = cst.tile([50, GM * 96], BF16, tag=f"fbt{g}")
            eng.dma_start(t[:].rearrange("p (m i) -> p m i", i=96),
                          d_fbt.ap()[:, GM * g:GM * (g + 1), :])
            fbt_g[g] = t

        def mpnn_dma(g, eng):
            mpnn_dma_fb(g, eng)
            t = cst.tile([96, GM * 96], BF16, tag=f"abt{g}")
            eng.dma_start(t[:].rearrange("p (m i) -> p m i", i=96),
                          d_abt.ap()[:, GM * g:GM * (g + 1), :])
            abt_g[g] = t
            t = cst.tile([96, GM * 48], BF16, tag=f"aat{g}")
            eng.dma_start(t[:].rearrange("p (m i) -> p m i", i=48),
                          d_aat.ap()[:, GM * g:GM * (g + 1), :])
            aat_g[g] = t
            t = cst.tile([40, GM * 48], BF16, tag=f"cat1{g}")
            eng.dma_start(t[:].rearrange("p (m i) -> p m i", i=48),
                          d_cat1.ap()[:, GM * g:GM * (g + 1), :])
            cat1_g[g] = t

        # ACT queue: conv0/conv1 weights + the first protein samples so
        # the conv pipeline can start immediately.
        w0dr_t = cst.tile([100, 2 * 96], F8, tag="w0dr")
        nc.scalar.dma_start(
            w0dr_t[:].rearrange("p (b o) -> p b o", b=2), d_w0dr.ap())
        b0_t = const_tile(d_b0, [96, 1], F32, eng=nc.scalar)
        x0_bufs = []

        def pvt_dma(s):
            t = xp.tile([100, 2 * SEGA], F8, tag="x0", bufs=4)
            eng = nc.scalar if s % 2 == 0 else nc.sync
            eng.dma_start(t[:].rearrange("p (a s) -> p a s", a=2),
                          d_pvt[s].ap())
            x0_bufs.append(t)

        wi_t = const_tile(d_wi, [50, 200])
        mpnn_dma_fb(0, nc.sync)
        t0 = xp.tile([100, 2 * SEGA], F8, tag="x0", bufs=4)
        nc.scalar.dma_start(
            t0[0:50, :].rearrange("p (a s) -> p a s", a=2),
            d_pvt[0].ap()[0:50, :, :])
        nc.sync.dma_start(
            t0[50:100, :].rearrange("p (a s) -> p a s", a=2),
            d_pvt[0].ap()[50:100, :, :])
        x0_bufs.append(t0)
        pvt_dma(1)
        abt0 = cst.tile([96, GM * 96], BF16, tag="abt0")
        nc.sync.dma_start(abt0[:].rearrange("p (m i) -> p m i", i=96),
                          d_abt.ap()[:, 0:GM, :])
        abt_g[0] = abt0
        aat0 = cst.tile([96, GM * 48], BF16, tag="aat0")
        nc.sync.dma_start(aat0[:].rearrange("p (m i) -> p m i", i=48),
                          d_aat.ap()[:, 0:GM, :])
        aat_g[0] = aat0
        cat10 = cst.tile([40, GM * 48], BF16, tag="cat10")
        nc.sync.dma_start(cat10[:].rearrange("p (m i) -> p m i", i=48),
                          d_cat1.ap()[:, 0:GM, :])
        cat1_g[0] = cat10
        w1dr_t = cst.tile([96, 3 * 2 * 128], F8, tag="w1dr")
        nc.scalar.dma_start(
            w1dr_t[:].rearrange("p (a b o) -> p a b o", a=3, b=2), d_w1dr.ap())
        b1_t = const_tile(d_b1, [128, 1], F32, eng=nc.scalar)
        pvt_dma(2)
        w2a_t = cst.tile([128, 4 * 2 * 128], F8, tag="w2a")
        nc.scalar.dma_start(
            w2a_t[:].rearrange("p (a b o) -> p a b o", a=4, b=2), d_w2a.ap())
        w2b_t = cst.tile([128, 4 * 2 * 80], F8, tag="w2b")
        nc.scalar.dma_start(
            w2b_t[:].rearrange("p (a b o) -> p a b o", a=4, b=2), d_w2b.ap())
        b2a_t = const_tile(d_b2a, [128, 1], F32, eng=nc.scalar)
        b2b_t = const_tile(d_b2b, [72, 1], F32, eng=nc.scalar)

        # Late-needed weights (W_o, FC, groups 2-3) ride the slower
        # gpsimd SWDGE so the two HWDGE queues stay clear for pvt.
        wh1_t = const_tile(d_wh1, [128, 200])
        wh2_t = const_tile(d_wh2, [72, 200])
        mpnn_dma(1, nc.sync)
        wo1_t = const_tile(d_wo1, [40, 200], eng=nc.gpsimd)
        wo2_t = const_tile(d_wo2, [128, 200], eng=nc.gpsimd)
        wo3_t = const_tile(d_wo3, [72, 200], eng=nc.gpsimd)
        mpnn_dma(2, nc.gpsimd)
        mpnn_dma(3, nc.gpsimd)
        fc0_t = [const_tile(d, [dim, 200], eng=nc.gpsimd) for d, dim in
                 zip(d_fc0, (128, 72, 128, 72))]
        fc0ba_t = const_tile(d_fc0ba, [128, 1], F32, eng=nc.gpsimd)
        fc0bb_t = const_tile(d_fc0bb, [72, 1], F32, eng=nc.gpsimd)
        fc1a_t = const_tile(d_fc1a, [128, 100], eng=nc.gpsimd)
        fc1b_t = const_tile(d_fc1b, [72, 100], eng=nc.gpsimd)
        fc1bias_t = const_tile(d_fc1bias, [100, 1], F32, eng=nc.gpsimd)
        fc2w_t = const_tile(d_fc2w, [100, 1], eng=nc.gpsimd)
        fc2b_t = const_tile(d_fc2b, [1, 1], F32, eng=nc.gpsimd)

        # static outputs of the two towers, feature-major [feat, M]
        embT1 = sbs.tile([128, M], BF16, tag="embT1")
        embT2 = sbs.tile([72, M], BF16, tag="embT2")
        prT1p = sbs.tile([128, M], F32, tag="prT1p")
        prT2p = sbs.tile([72, M], F32, tag="prT2p")
        prT1 = sbs.tile([128, M], BF16, tag="prT1")
        prT2 = sbs.tile([72, M], BF16, tag="prT2")

        # ================= per-molecule MPNN (bf16) =================
        msg_t, nt_t = {}, {}
        natg_t = {}

        binp_t = {}

        def emit_binput(m):
            g, r = m // GM, m % GM
            fb_m = fbt_g[g][:, r * 96:(r + 1) * 96]
            ps = pp.tile([96, 200], F32, tag="mp", bufs=3)
            nc.tensor.matmul(ps[:], fb_m, wi_t[:], start=True, stop=True)
            binp = sbs.tile([96, 200], BF16, tag=f"binp{m}")
            nc.scalar.copy(binp[:], ps[:])
            msg = sbs.tile([96, 200], BF16, tag=f"msg{m}")
            nc.vector.tensor_scalar(msg[:], ps[:], 0.0, None, op0=ALU.max)
            msg_t[m] = msg
            binp_t[m] = binp

        def emit_iter_nei(m):
            # nT[h, i] = sum_j msg[j, h] * ab[j, i], feature-major halves
            # packed into one PSUM bank, one fused copy out.
            g, r = m // GM, m % GM
            ab_m = abt_g[g][:, r * 96:(r + 1) * 96]
            msg = msg_t[m]
            psn = pp.tile([128, 192], F32, tag="tp", bufs=2)
            nc.tensor.matmul(psn[0:128, 0:96], msg[:, 0:128], ab_m,
                             start=True, stop=True)
            nc.tensor.matmul(psn[0:72, 96:192], msg[:, 128:200], ab_m,
                             start=True, stop=True, skip_group_check=True)
            nt = tmp.tile([128, 192], BF16, tag="nt", bufs=4)
            nc.vector.tensor_copy(nt[0:128, 0:96], psn[0:128, 0:96])
            nc.scalar.copy(nt[0:72, 96:192], psn[0:72, 96:192])
            nt_t[m] = nt

        def emit_iter_upd(m):
            # msg = relu(nT.T @ W_h + binput); the binput add runs on
            # the DVE (PE is the bottleneck, DVE/ACT have slack)
            nt = nt_t.pop(m)
            msg = msg_t[m]
            psH = pp.tile([96, 200], F32, tag="mp", bufs=3)
            nc.tensor.matmul(psH[:], nt[0:128, 0:96], wh1_t[:],
                             start=True, stop=False)
            nc.tensor.matmul(psH[:], nt[0:72, 96:192], wh2_t[:],
                             start=False, stop=True)
            tm = tmp.tile([96, 200], F32, tag="mtmp", bufs=3)
            nc.vector.tensor_add(tm[:], psH[:], binp_t[m][:])
            nc.scalar.activation(msg[:], tm[:], AF.Relu)

        def emit_atom_pre(m):
            g, r = m // GM, m % GM
            aa_m = aat_g[g][:, r * 48:(r + 1) * 48]
            msg = msg_t.pop(m)
            binp_t.pop(m, None)
            if r == 0:
                natg_new = tmp.tile([128, GM * 96], BF16, tag="natg", bufs=2)
                natg_t[g] = natg_new
            natg = natg_t[g]
            pst = pp.tile([128, 96], F32, tag="tp", bufs=2)
            nc.tensor.matmul(pst[0:128, 0:48], msg[:, 0:128], aa_m,
                             start=True, stop=True)
            nc.tensor.matmul(pst[0:72, 48:96], msg[:, 128:200], aa_m,
                             start=True, stop=True, skip_group_check=True)
            nc.scalar.copy(natg[:, r * 96:(r + 1) * 96], pst[:])

        def emit_atom_group(g):
            natg = natg_t.pop(g)
            natv = natg[:].rearrange("p (m i) -> p m i", i=96)
            c1_g = cat1_g[g]
            for lo, hi, wsl, emb, scr in (
                    (0, 128, slice(0, 128), embT1, "ah1"),
                    (128, 200, slice(128, 200), embT2, "ah2")):
                n = hi - lo
                psU = pp.tile([n, GM * 48], F32, tag="mp", bufs=3)
                nc.tensor.matmul(psU[:], wo1_t[:, wsl], c1_g[:],
                                 start=True, stop=False)
                nc.tensor.matmul(
                    psU[:].rearrange("p (m i) -> p m i", i=48),
                    wo2_t[:, wsl], natv[0:128, :, 0:48],
                    start=False, stop=False)
                nc.tensor.matmul(
                    psU[:].rearrange("p (m i) -> p m i", i=48),
                    wo3_t[:, wsl], natv[0:72, :, 48:96],
                    start=False, stop=True)
                ah = tmp.tile([n, GM * 48], BF16, tag=scr, bufs=2)
                nc.scalar.activation(ah[:], psU[:], AF.Relu)
                red = tmp.tile([n, GM], F32, tag=scr + "r", bufs=2)
                nc.vector.reduce_sum(
                    red[:], ah[:].rearrange("p (m a) -> p m a", a=48), axis=AX.X)
                nc.scalar.mul(emb[:, GM * g:GM * (g + 1)], red[:], 1.0 / 48)

        # ================= per-sample protein conv tower =================
        x1_t, x2_t = {}, {}

        def dup_ap(t, nparts, off):
            # [nparts, 2, NCH] DR pair AP over a duplicated [nparts,
            # 2*SEGA] tile (slot1 = same data shifted left one column).
            a = t[:, off:off + NCH]
            return bass.AP(a.tensor, a.offset,
                           [[2 * SEGA, nparts], [SEGA, 2], [1, NCH]])

        def dup_write(t, nparts, off, ps, bias, scale):
            # ACT writes slot0 = relu(scale*psum + bias) in fp8; DVE
            # copies the shifted byte range into slot1.
            nc.scalar.activation(t[:, off:off + NCH], ps[:], AF.Relu,
                                 bias=bias, scale=scale)
            nc.vector.tensor_copy(t[:, SEGA + off - 1:SEGA + off - 1 + NCH],
                                  t[:, off - 1:off - 1 + NCH])

        def emit_conv0(s):
            if s + 3 < M:
                pvt_dma(s + 3)
            x0 = x0_bufs[s]
            x1 = xp.tile([96, 2 * SEGA], F8, tag="x1", bufs=3)
            x1w = x1[:].bitcast(F32)
            nc.gpsimd.memset(x1w[:, 0:1], 0.0)
            nc.gpsimd.memset(x1w[:, 250:253], 0.0)
            nc.gpsimd.memset(x1w[:, 502:504], 0.0)
            w0v = w0dr_t[:].rearrange("p (b o) -> p b o", b=2)
            for c in range(2):
                off = PAD + c * NCH
                a = x0[:, 2 + c * NCH:2 + c * NCH + NCH]
                rhs = bass.AP(a.tensor, a.offset,
                              [[2 * SEGA, 100], [SEGA, 2], [1, NCH]])
                ps = pp.tile([96, NCH], F32, tag="cv", bufs=3)
                nc.tensor.matmul(ps[:], w0v, rhs, start=True, stop=True,
                                 perf_mode=DR)
                dup_write(x1, 96, off, ps, b0_t[:], S1)
            x1_t[s] = x1

        def emit_conv1(s):
            x1 = x1_t.pop(s)
            x2 = xp.tile([128, 2 * SEGA], F8, tag="x2", bufs=3)
            x2w = x2[:].bitcast(F32)
            nc.gpsimd.memset(x2w[:, 0:1], 0.0)
            nc.gpsimd.memset(x2w[:, 250:253], 0.0)
            nc.gpsimd.memset(x2w[:, 502:504], 0.0)
            w1v = w1dr_t[:].rearrange("p (a b o) -> p a b o", a=3, b=2)
            for c in range(2):
                off = PAD + c * NCH
                ps = pp.tile([128, NCH], F32, tag="cv", bufs=3)
                for b in range(3):
                    nc.tensor.matmul(ps[:], w1v[:, b, :, :],
                                     dup_ap(x1, 96, off + 2 * b - 2),
                                     start=(b == 0), stop=(b == 2),
                                     perf_mode=DR)
                dup_write(x2, 128, off, ps, b1_t[:], S2)
            x2_t[s] = x2

        def emit_conv2(s):
            x2 = x2_t.pop(s)
            mxA = tmp.tile([128, 2], F32, tag="mxA", bufs=3)
            mxB = tmp.tile([72, 2], F32, tag="mxB", bufs=3)
            w2av = w2a_t[:].rearrange("p (a b o) -> p a b o", a=4, b=2)
            w2bv = w2b_t[:].rearrange("p (a b o) -> p a b o", a=4, b=2)
            for c in range(2):
                off = PAD + c * NCH
                psA = pp.tile([128, NCH], F32, tag="cv", bufs=3)
                for b in range(4):
                    nc.tensor.matmul(psA[:], w2av[:, b, :, :],
                                     dup_ap(x2, 128, off + 2 * b - 3),
                                     start=(b == 0), stop=(b == 3),
                                     perf_mode=DR)
                nc.vector.reduce_max(mxA[:, c:c + 1], psA[:], axis=AX.X)
                psB = pp.tile([80, NCH], F32, tag="cv", bufs=3)
                for b in range(4):
                    nc.tensor.matmul(psB[:], w2bv[:, b, :, :],
                                     dup_ap(x2, 128, off + 2 * b - 3),
                                     start=(b == 0), stop=(b == 3),
                                     perf_mode=DR)
                nc.vector.reduce_max(mxB[:, c:c + 1], psB[0:72, :], axis=AX.X)
            nc.vector.reduce_max(prT1p[:, s:s + 1], mxA[:], axis=AX.X)
            nc.vector.reduce_max(prT2p[:, s:s + 1], mxB[:], axis=AX.X)
            nc.scalar.activation(prT1[:, s:s + 1], prT1p[:, s:s + 1],
                                 AF.Relu, bias=b2a_t[:], scale=S3)
            nc.scalar.activation(prT2[:, s:s + 1], prT2p[:, s:s + 1],
                                 AF.Relu, bias=b2b_t[:], scale=S3)

        # ============== software-pipelined emission schedule ==============
        # slot k: conv0(k+1) | conv1(k) | conv2(k-1) contiguous, with two
        # MPNN microstages of group k//4 around the conv run.
        def mp_stage(g, idx):
            mols = [GM * g + r for r in range(GM)]
            if idx == 0:
                for m in mols:
                    emit_binput(m)
            elif idx in (1, 3):
                for m in mols:
                    emit_iter_nei(m)
            elif idx in (2, 4):
                for m in mols:
                    emit_iter_upd(m)
            elif idx == 5:
                for m in mols:
                    emit_atom_pre(m)
            elif idx == 6:
                emit_atom_group(g)

        # PE clock warmup: the tensor engine p-state ramps with busy
        # time, and the first ~12us are DMA-bound with an idle PE. Run
        # throwaway matmuls (one PSUM allocation, pure in-engine WAW) so
        # the clock is ramped when real work arrives.
        # warm with the same fp8 DoubleRow profile as the real conv work
        # so the clock governor's conservative period burns during the
        # DMA wait instead of on the first samples.
        wu8 = wu[:].bitcast(F8)
        wuw = bass.AP(wu8.tensor, wu8.offset,
                      [[1024, 128], [128, 2], [1, 128]])
        wur = bass.AP(wu8.tensor, wu8.offset,
                      [[1024, 128], [512, 2], [1, 500]])
        pswu = pp.tile([128, 500], F32, tag="cv", bufs=3)
        for _ in range(13):
            nc.tensor.matmul(pswu[:], wuw, wur, start=True, stop=True,
                             perf_mode=DR)

        def keep_warm(n):
            # standalone weight loads: PE-busy filler (keeps the clock
            # p-state up) with no PSUM or cross-engine dependencies.
            for _ in range(n):
                nc.tensor.ldweights(wu[:, 0:128])

        mp_stage(0, 0)
        emit_conv0(0)
        keep_warm(10)
        fc_state = {}

        def emit_fc_early():
            # emb-dependent half of fc0 right after the last atom group;
            # the prT half joins at the very end.
            ps0a = pp.tile([128, M], F32, tag="mp", bufs=3)
            nc.tensor.matmul(ps0a[:], fc0_t[0][:, 0:128], embT1[:],
                             start=True, stop=False)
            nc.tensor.matmul(ps0a[:], fc0_t[1][:, 0:128], embT2[:],
                             start=False, stop=False)
            ps0b = pp.tile([72, M], F32, tag="mp", bufs=3)
            nc.tensor.matmul(ps0b[:], fc0_t[0][:, 128:200], embT1[:],
                             start=True, stop=False)
            nc.tensor.matmul(ps0b[:], fc0_t[1][:, 128:200], embT2[:],
                             start=False, stop=False)
            fc_state["a"] = ps0a
            fc_state["b"] = ps0b

        for k in range(M):
            g, phase = k // GM, k % GM
            if k + 1 < M:
                emit_conv0(k + 1)
            mp_stage(g, 2 * phase + 1)
            if k < 2:
                keep_warm(8)
            emit_conv1(k)
            if k - 1 >= 0:
                emit_conv2(k - 1)
            if 2 * phase + 2 <= 6:
                mp_stage(g, 2 * phase + 2)
            if phase == GM - 1 and g + 1 < 4:
                mp_stage(g + 1, 0)
            if k == M - 1:
                emit_fc_early()
        emit_conv2(M - 1)

        # ================= FC head (prT half + tail) =================
        ps0a, ps0b = fc_state.pop("a"), fc_state.pop("b")
        nc.tensor.matmul(ps0a[:], fc0_t[2][:, 0:128], prT1[:],
                         start=False, stop=False)
        nc.tensor.matmul(ps0a[:], fc0_t[3][:, 0:128], prT2[:],
                         start=False, stop=True)
        h0a = tmp.tile([128, M], BF16, tag="h0a")
        nc.scalar.activation(h0a[:], ps0a[:], AF.Relu, bias=fc0ba_t[:])
        nc.tensor.matmul(ps0b[:], fc0_t[2][:, 128:200], prT1[:],
                         start=False, stop=False)
        nc.tensor.matmul(ps0b[:], fc0_t[3][:, 128:200], prT2[:],
                         start=False, stop=True)
        h0b = tmp.tile([72, M], BF16, tag="h0b")
        nc.scalar.activation(h0b[:], ps0b[:], AF.Relu, bias=fc0bb_t[:])

        ps1 = pp.tile([100, M], F32, tag="tp", bufs=2)
        nc.tensor.matmul(ps1[:], fc1a_t[:], h0a[:], start=True, stop=False)
        nc.tensor.matmul(ps1[:], fc1b_t[:], h0b[:], start=False, stop=True)
        h1 = tmp.tile([100, M], BF16, tag="h1")
        nc.scalar.activation(h1[:], ps1[:], AF.Relu, bias=fc1bias_t[:])

        ps2 = pp.tile([1, M], F32, tag="tp", bufs=2)
        nc.tensor.matmul(ps2[:], fc2w_t[:], h1[:], start=True, stop=True)
        outsb = tmp.tile([1, M], F32, tag="outsb")
        nc.scalar.add(outsb[:], ps2[:], fc2b_t[:, 0:1])
        nc.sync.dma_start(d_out.ap(), outsb[:])

    nc.compile()
    return nc


def _prep(inputs):
    """Host preprocessing: returns the 8 per-core in_maps."""
    import ml_dtypes
    f32 = np.float32
    bf16 = ml_dtypes.bfloat16
    f8 = ml_dtypes.float8_e4m3fn

    fatoms = np.asarray(inputs["fatoms"], f32)
    fbonds = np.asarray(inputs["fbonds"], f32)
    agraph = np.asarray(inputs["agraph"])
    bgraph = np.asarray(inputs["bgraph"])
    pseq = np.asarray(inputs["protein_seq"])
    W_i = np.asarray(inputs["W_i"], f32)
    W_h = np.asarray(inputs["W_h"], f32)
    W_o_w = np.asarray(inputs["W_o_w"], f32)
    W_o_b = np.asarray(inputs["W_o_b"], f32)
    embp = np.asarray(inputs["embed_protein"], f32)

    # protein embeddings, channel-major, pre-scaled for fp8
    pvT = np.ascontiguousarray((embp * A0)[pseq].transpose(0, 2, 1))  # (B,50,L)

    # adjacency one-hots (counts; contraction-dim-major for lhsT/rhs use)
    ar = np.arange(B)[:, None, None]
    cntB = np.zeros((B, NB, NB), f32)
    np.add.at(cntB, (ar, np.arange(NB)[None, :, None], bgraph), 1.0)
    abt = np.ascontiguousarray(cntB.transpose(0, 2, 1))        # (B, j, i)
    cntA = np.zeros((B, NA, NB), f32)
    np.add.at(cntA, (ar, np.arange(NA)[None, :, None], agraph), 1.0)
    aat = np.ascontiguousarray(cntA.transpose(0, 2, 1))        # (B, j, a)

    fbT = fbonds.transpose(0, 2, 1)                            # (B, 50, 96)
    faT = fatoms.transpose(0, 2, 1)                            # (B, 39, 48)
    cat1 = np.concatenate([faT, np.ones((B, 1, NA), f32)], axis=1)  # (B,40,48)

    conv_w = [np.asarray(inputs[f"conv{i}_w"], f32) for i in range(3)]
    conv_b = [np.asarray(inputs[f"conv{i}_b"], f32) for i in range(3)]

    # conv0 as one DoubleRow block: pair rows 0-49 = (tap0 -> tap1),
    # rows 50-99 = (tap2 -> zero)
    w0 = conv_w[0] * G0                                        # (96, 50, 3)
    w0dr = np.zeros((100, 2, 96), f32)
    w0dr[0:50, 0, :] = w0[:, :, 0].T
    w0dr[50:100, 0, :] = w0[:, :, 2].T
    w0dr[0:50, 1, :] = w0[:, :, 1].T

    def dr_pack(w, nblk, O):
        # w: (Ow, C, K) -> (C, nblk, 2, O) blocks of tap pairs; O may be
        # zero-padded past Ow so the DR pair stride stays 16-aligned
        Ow, C, K = w.shape
        out = np.zeros((C, nblk, 2, O), f32)
        for b in range(nblk):
            for i in range(2):
                t = 2 * b + i
                if t < K:
                    out[:, b, i, 0:Ow] = w[:, :, t].T
        return out

    w1dr = dr_pack(conv_w[1] * G1, 3, 128)                     # (96,3,2,128)
    w2 = conv_w[2] * G2                                        # (200,128,7)
    w2adr = dr_pack(w2[0:128], 4, 128)                         # (128,4,2,128)
    w2bdr = dr_pack(w2[128:200], 4, 80)        # zero-padded 72->80 for DR

    fcw = [np.asarray(inputs[f"fc{i}_w"], f32) for i in range(3)]
    fcb = [np.asarray(inputs[f"fc{i}_b"], f32) for i in range(3)]

    wo1 = np.zeros((40, 200), f32)
    wo1[:39] = W_o_w[0:39]
    wo1[39] = W_o_b

    shared = {
        "wi": W_i.astype(bf16),
        "wh1": W_h[0:128].astype(bf16), "wh2": W_h[128:200].astype(bf16),
        "wo1": wo1.astype(bf16),
        "wo2": W_o_w[39:167].astype(bf16), "wo3": W_o_w[167:239].astype(bf16),
        "w0dr": w0dr.astype(f8),
        "b0": (conv_b[0] * A1).reshape(96, 1).astype(f32),
        "w1dr": w1dr.astype(f8),
        "b1": (conv_b[1] * A2).reshape(128, 1).astype(f32),
        "w2a": w2adr.astype(f8), "w2b": w2bdr.astype(f8),
        "b2a": conv_b[2][0:128].reshape(128, 1).astype(f32),
        "b2b": conv_b[2][128:200].reshape(72, 1).astype(f32),
        "fc0a": fcw[0][0:128].astype(bf16),
        "fc0b": fcw[0][128:200].astype(bf16),
        "fc0c": fcw[0][200:328].astype(bf16),
        "fc0d": fcw[0][328:400].astype(bf16),
        "fc0ba": fcb[0][0:128].reshape(128, 1).astype(f32),
        "fc0bb": fcb[0][128:200].reshape(72, 1).astype(f32),
        "fc1a": fcw[1][0:128].astype(bf16),
        "fc1b": fcw[1][128:200].astype(bf16),
        "fc1bias": fcb[1].reshape(100, 1).astype(f32),
        "fc2w": fcw[2].astype(bf16),
        "fc2b": fcb[2].reshape(1, 1).astype(f32),
    }
    shared = {k: np.ascontiguousarray(v) for k, v in shared.items()}

    # protein activations for the DR conv0: slot0 rows 0-49 = x0 with
    # x[l] at col 3+l (tap0 reads col 2+n -> x[n-1]); rows 50-99 at col
    # 1+l (-> x[n+1], tap2). slot1 = slot0 shifted left one column.
    pvt_pad = np.zeros((B, 100, 2, SEGA), f8)
    pv8 = pvT.astype(f8)
    pvt_pad[:, 0:50, 0, 3:3 + L] = pv8
    pvt_pad[:, 50:100, 0, 1:1 + L] = pv8
    pvt_pad[:, 0:50, 1, 2:2 + L] = pv8
    pvt_pad[:, 50:100, 1, 0:0 + L] = pv8

    in_maps = []
    for c in range(NCORES):
        lo = c * M
        im = dict(shared)
        for s in range(M):
            im[f"pvt{s}"] = np.ascontiguousarray(pvt_pad[lo + s])
        im["fbt"] = np.ascontiguousarray(
            fbT[lo:lo + M].transpose(1, 0, 2)).astype(bf16)
        im["cat1"] = np.ascontiguousarray(
            cat1[lo:lo + M].transpose(1, 0, 2)).astype(bf16)
        im["abt"] = np.ascontiguousarray(
            abt[lo:lo + M].transpose(1, 0, 2)).astype(bf16)
        im["aat"] = np.ascontiguousarray(
            aat[lo:lo + M].transpose(1, 0, 2)).astype(bf16)
        in_maps.append(im)
    return in_maps


def get_nc():
    if "nc" not in _CACHE:
        _CACHE["nc"] = _build_nc()
    return _CACHE["nc"]


def kernel(**inputs) -> np.ndarray:
    nc = get_nc()
    in_maps = _prep(inputs)
    res = run_bass_kernel_spmd(nc, in_maps, core_ids=list(range(NCORES)))
    outs = [res.results[c]["out"].reshape(M, 1) for c in range(NCORES)]
    return np.concatenate(outs, axis=0).astype(np.float32)
